# revision 1
# baseline (speedup 1.0000x reference)
"""Multi-head attention (B=8, T=2048, D=512, H=8) on 8 TRN2 NeuronCores.

Sharding: data-parallel over batch — one batch element per core, no
collectives. Host-side prep (part of shard/unshard): transpose x inputs to
[D, T], cast matmul operands to bf16, pass (1 - mask)^T chunk-major, and
transpose the per-core output y^T back to [T, D].

Per-core algorithm ("transposed flash", everything in one PE tiling mode):
  P1: Q^T = Wq x^T, K^T = Wk x^T (padded per-head into zero-padded 128-row
      tiles), V = x Wv^T (augmented with a ones column per head for the
      softmax denominator).
  P2: per (q-block, head, t2-chunk):
        S^T[t2,q] = Kpad_h^T.T @ Q^T          (PSUM, scale deferred)
        P_raw     = exp(S^T / 8)              (ScalarE, PSUM -> SBUF bf16)
        P         = P_raw * (1-mask)^T        (VectorE; equals reference's
                                               where(mask,-inf) + where(mask,0)
                                               since exp(-1e4) == 0 in f32)
        O_aug^T  += Vaug_h.T @ P              (PSUM accum; row 64 = denom)
      epilogue: recip(denom) -> broadcast -> O^T = O_aug^T[0:64] * recip.
  P3: y^T = Wo^T.T @ O^T (+bo), DMA out.

Biases bq, bk, bo are applied (per-partition fused adds); bv via a
broadcast add on V eviction. No max-subtraction in softmax: scores are
O(6) so exp is safe in f32, matching the reference to ~bf16 accuracy.
"""

import numpy as np
import ml_dtypes

B, T, FDIM, H = 8, 2048, 512, 8
DK = FDIM // H          # 64
NFT = FDIM // 128       # 4 fo-tiles
NCH = T // 128          # 16 t2-chunks
QB = 2                  # q blocks
QBS = T // QB           # 1024
N_CORES = 8

BF16 = ml_dtypes.bfloat16

_cache = {}


def _build_nc():
    import concourse.bass as bass
    import concourse.mybir as mybir
    from concourse import bacc, tile

    f32 = mybir.dt.float32
    bf16 = mybir.dt.bfloat16
    Exp = mybir.ActivationFunctionType.Exp
    Alu = mybir.AluOpType

    nc = bacc.Bacc("TRN2", target_bir_lowering=False, debug=False,
                   num_devices=N_CORES)

    # DRAM I/O (per-core shard shapes)
    xqT = nc.dram_tensor("xqT", [FDIM, T], bf16, kind="ExternalInput")
    xkT = nc.dram_tensor("xkT", [FDIM, T], bf16, kind="ExternalInput")
    xvT = nc.dram_tensor("xvT", [FDIM, T], bf16, kind="ExternalInput")
    wqT = nc.dram_tensor("wqT", [FDIM, FDIM], bf16, kind="ExternalInput")
    wkT = nc.dram_tensor("wkT", [FDIM, FDIM], bf16, kind="ExternalInput")
    wvT = nc.dram_tensor("wvT", [FDIM, FDIM], bf16, kind="ExternalInput")
    woT = nc.dram_tensor("woT", [FDIM, FDIM], bf16, kind="ExternalInput")
    bq = nc.dram_tensor("bq", [FDIM], f32, kind="ExternalInput")
    bk = nc.dram_tensor("bk", [FDIM], f32, kind="ExternalInput")
    bv = nc.dram_tensor("bv", [FDIM], f32, kind="ExternalInput")
    bo = nc.dram_tensor("bo", [FDIM], f32, kind="ExternalInput")
    mbar = nc.dram_tensor("mbar", [NCH, 128, T], bf16, kind="ExternalInput")
    yT = nc.dram_tensor("yT", [FDIM, T], f32, kind="ExternalOutput")
    # DRAM bounce rows for partition-broadcasting softmax reciprocals
    rscratch = nc.dram_tensor("rscratch", [QB * H, QBS], f32)

    import os
    dbg = os.environ.get("KERNEL_DEBUG_DUMPS") == "1"
    if dbg:
        dbg_qt = nc.dram_tensor("dbg_qt", [128, T], bf16, kind="ExternalOutput")
        dbg_kp = nc.dram_tensor("dbg_kp", [2, 128, T], bf16, kind="ExternalOutput")
        dbg_va = nc.dram_tensor("dbg_va", [128, H * (DK + 1)], bf16, kind="ExternalOutput")
        dbg_pm = nc.dram_tensor("dbg_pm", [128, QBS], bf16, kind="ExternalOutput")
        dbg_osb = nc.dram_tensor("dbg_osb", [64, QBS], bf16, kind="ExternalOutput")
        dbg_rb = nc.dram_tensor("dbg_rb", [2, QBS], f32, kind="ExternalOutput")

    _dma_rr = [0]

    with tile.TileContext(nc) as tc:
        def bulk_dma(out, in_):
            nc.sync.dma_start(out=out, in_=in_)

        with (
            tc.tile_pool(name="consts", bufs=1) as consts,
            tc.tile_pool(name="qt", bufs=1) as qt_pool,
            tc.tile_pool(name="kpad", bufs=1) as kpad_pool,
            tc.tile_pool(name="vaug", bufs=1) as vaug_pool,
            tc.tile_pool(name="osb", bufs=1) as osb_pool,
            tc.tile_pool(name="ysb", bufs=1) as ysb_pool,
        ):
            # ---- consts: weights + biases ----
            wq_sb = [consts.tile([128, FDIM], bf16, tag=f"wq{fc}", name=f"wq{fc}") for fc in range(4)]
            wk_sb = [consts.tile([128, FDIM], bf16, tag=f"wk{fc}", name=f"wk{fc}") for fc in range(4)]
            wv_sb = [consts.tile([128, FDIM], bf16, tag=f"wv{fc}", name=f"wv{fc}") for fc in range(4)]
            wo_sb = [consts.tile([128, FDIM], bf16, tag=f"wo{j}", name=f"wo{j}") for j in range(NFT)]
            # wv first: the V projection is the head of the critical path
            for fc in range(4):
                bulk_dma(out=wv_sb[fc][:], in_=wvT[fc * 128:(fc + 1) * 128, :])
            for fc in range(4):
                bulk_dma(out=wq_sb[fc][:], in_=wqT[fc * 128:(fc + 1) * 128, :])
                bulk_dma(out=wk_sb[fc][:], in_=wkT[fc * 128:(fc + 1) * 128, :])
            for j in range(NFT):
                bulk_dma(out=wo_sb[j][:], in_=woT[j * 128:(j + 1) * 128, :])

            bq_sb = consts.tile([128, NFT], f32, tag="bq", name="bq")
            bk_sb = consts.tile([128, NFT], f32, tag="bk", name="bk")
            bo_sb = consts.tile([128, NFT], f32, tag="bo", name="bo")
            for b_dram, b_t in ((bq, bq_sb), (bk, bk_sb), (bo, bo_sb)):
                nc.sync.dma_start(out=b_t[:], in_=b_dram.ap().rearrange("(j p) -> p j", p=128))
            bv_bcast = consts.tile([128, FDIM], f32, tag="bv_bcast", name="bv_bcast")
            nc.sync.dma_start(
                out=bv_bcast[:],
                in_=bv.ap().rearrange("(a f) -> a f", a=1).to_broadcast([128, FDIM]))

            # ---- persistent activation tiles ----
            qT_sb = [qt_pool.tile([128, T], bf16, tag=f"qT{j}", name=f"qT{j}") for j in range(NFT)]
            kpad = [kpad_pool.tile([128, T], bf16, tag=f"kp{h}", name=f"kp{h}") for h in range(H)]
            # zero the unused head-half of each padded K tile, once
            for h in range(H):
                half = slice(64, 128) if h % 2 == 0 else slice(0, 64)
                nc.vector.memset(kpad[h][half, :], 0.0)
            vaug = [vaug_pool.tile([128, H * (DK + 1)], bf16, tag=f"va{tt}", name=f"va{tt}")
                    for tt in range(NCH)]
            # ones column per head in V_aug
            for tt in range(NCH):
                va = vaug[tt][:].rearrange("p (h d) -> p h d", d=DK + 1)
                nc.vector.memset(va[:, :, DK:DK + 1], 1.0)

            o2_sb = {}
            for qb in range(QB):
                for j in range(NFT):
                    o2_sb[(qb, j)] = osb_pool.tile([128, QBS], bf16, tag=f"o2_{qb}_{j}",
                                                   name=f"o2_{qb}_{j}")

            # ============ PSUM pool (8 banks total, shared by phases) ======
            # tag "s":  2 x [128,1024] f32 = 4 banks  (scores / QK-proj)
            # tag o0/o1: 1 x [128,1024]-sized slot each = 4 banks
            #            (attnV accumulators, V-proj, P3 y-tiles)
            with (
                tc.tile_pool(name="xt", bufs=5) as xt_pool,
                tc.tile_pool(name="mask", bufs=16) as mask_pool,
                tc.tile_pool(name="praw", bufs=3) as praw_pool,
                tc.tile_pool(name="pm", bufs=3) as pm_pool,
                tc.tile_pool(name="rb", bufs=1) as rb_pool,
                tc.tile_pool(name="psum", bufs=2, space="PSUM") as psum_pool,
            ):
                def load_xT(xT_dram, tag):
                    tiles = []
                    for fc in range(4):
                        xt = xt_pool.tile([128, T], bf16, tag=tag, bufs=4, name="xt")
                        bulk_dma(out=xt[:], in_=xT_dram[fc * 128:(fc + 1) * 128, :])
                        tiles.append(xt)
                    return tiles

                def v_proj_tile(tt, ptag):
                    if True:
                        ps = psum_pool.tile([128, 512], mybir.dt.float32,
                                            tag=ptag, bufs=1, name="vp")
                        for fc in range(4):
                            nc.tensor.matmul(
                                ps[:],
                                xts_v[fc][:, tt * 128:(tt + 1) * 128],
                                wv_sb[fc][:],
                                start=(fc == 0), stop=(fc == 3),
                            )
                        va = vaug[tt][:].rearrange("p (h d) -> p h d", d=DK + 1)
                        nc.vector.scalar_tensor_tensor(
                            out=va[:, :, 0:DK],
                            in0=ps[:].rearrange("p (h d) -> p h d", d=DK),
                            scalar=1.0,
                            in1=bv_bcast[:].rearrange("p (h d) -> p h d", d=DK),
                            op0=Alu.mult, op1=Alu.add,
                        )

                def proj_groups(j, xts, w_sb, b_t, dst, slices):
                    for s in slices:
                        ps = psum_pool.tile([128, 512], mybir.dt.float32,
                                            tag="s", name="qkp")
                        for fc in range(4):
                            nc.tensor.matmul(
                                ps[:],
                                w_sb[fc][:, j * 128:(j + 1) * 128],
                                xts[fc][:, s * 512:(s + 1) * 512],
                                start=(fc == 0), stop=(fc == 3),
                            )
                        sl = slice(s * 512, (s + 1) * 512)
                        if dst is not None:
                            nc.vector.tensor_scalar_add(dst[:, sl], ps[:],
                                                        b_t[:, j:j + 1])
                        else:  # K: evict into the two padded per-head tiles
                            nc.vector.tensor_scalar_add(
                                kpad[2 * j][0:64, sl], ps[0:64, :],
                                b_t[0:64, j:j + 1])
                            nc.vector.tensor_scalar_add(
                                kpad[2 * j + 1][64:128, sl], ps[64:128, :],
                                b_t[64:128, j:j + 1])

                def q_proj(j, half):
                    proj_groups(j, xts_q, wq_sb, bq_sb, qT_sb[j],
                                range(2 * half, 2 * half + 2))

                def k_proj(j, half):
                    proj_groups(j, xts_k, wk_sb, bk_sb, None,
                                range(2 * half, 2 * half + 2))

                def p3(qb):
                    qsl = slice(qb * QBS, (qb + 1) * QBS)
                    for i in range(NFT):
                        y_ps = psum_pool.tile([128, QBS], mybir.dt.float32,
                                              tag=f"o{i % 2}", bufs=1, name="y")
                        for j in range(NFT):
                            for s in range(2):
                                nc.tensor.matmul(
                                    y_ps[:, s * 512:(s + 1) * 512],
                                    wo_sb[j][:, i * 128:(i + 1) * 128],
                                    o2_sb[(qb, j)][:, s * 512:(s + 1) * 512],
                                    start=(j == 0), stop=(j == NFT - 1),
                                )
                        y_sb = ysb_pool.tile([128, QBS], mybir.dt.float32, tag="ysb", name="ysb")
                        nc.vector.tensor_scalar_add(y_sb[:], y_ps[:], bo_sb[:, i:i + 1])
                        nc.sync.dma_start(out=yT[i * 128:(i + 1) * 128, qsl], in_=y_sb[:])

                # ---- P1 prefix: V first (every head needs it), then QK j=0
                xts_v = load_xT(xvT, "xq")  # slots reused by xq after v_proj
                for tt in range(NCH):
                    v_proj_tile(tt, f"o{tt % 2}")
                xts_k = load_xT(xkT, "xk")
                xts_q = load_xT(xqT, "xq")
                k_proj(0, 0)
                k_proj(0, 1)
                q_proj(0, 0)

                if dbg:
                    nc.sync.dma_start(out=dbg_qt.ap(), in_=qT_sb[0][:])
                    nc.sync.dma_start(out=dbg_kp.ap()[0], in_=kpad[0][:])
                    nc.sync.dma_start(out=dbg_kp.ap()[1], in_=kpad[1][:])
                    nc.sync.dma_start(out=dbg_va.ap(), in_=vaug[0][:])

                # ---- P2 + P3, with remaining QK projections interleaved ----
                for qb in range(QB):
                    qsl = slice(qb * QBS, (qb + 1) * QBS)
                    mask_t = []
                    for c in range(NCH):
                        mt = mask_pool.tile([128, QBS], bf16, tag="mask", name="mask")
                        nc.sync.dma_start(out=mt[:], in_=mbar[c, :, qsl])
                        mask_t.append(mt)

                    for h in range(H):
                        j = h // 2
                        o_ps = psum_pool.tile([DK + 1, QBS], mybir.dt.float32,
                                              tag=f"o{h % 2}", bufs=1, name="o")
                        for c in range(NCH):
                            s_ps = psum_pool.tile([128, QBS], mybir.dt.float32,
                                                  tag="s", name="s")
                            for s in range(2):
                                nc.tensor.matmul(
                                    s_ps[:, s * 512:(s + 1) * 512],
                                    kpad[h][:, c * 128:(c + 1) * 128],
                                    qT_sb[j][:, qb * QBS + s * 512: qb * QBS + (s + 1) * 512],
                                    start=True, stop=True,
                                )
                            p_raw = praw_pool.tile([128, QBS], bf16, tag="praw", name="praw")
                            nc.scalar.activation(p_raw[:], s_ps[:], Exp,
                                                 bias=0.0, scale=0.125)
                            p_m = pm_pool.tile([128, QBS], bf16, tag="pm", name="pm")
                            nc.vector.tensor_mul(p_m[:], p_raw[:], mask_t[c][:])
                            if dbg and qb == 0 and h == 0 and c == 0:
                                nc.sync.dma_start(out=dbg_pm.ap(), in_=p_m[:])
                            for s in range(2):
                                nc.tensor.matmul(
                                    o_ps[:, s * 512:(s + 1) * 512],
                                    vaug[c][:, h * (DK + 1):(h + 1) * (DK + 1)],
                                    p_m[:, s * 512:(s + 1) * 512],
                                    start=(c == 0), stop=(c == NCH - 1),
                                )
                        # epilogue: divide by the denominator (row DK of o_ps).
                        # reciprocal is ~8 cyc/elem/lane -> split the [1,1024]
                        # row over 8 partitions via SBUF->SBUF DMA; a DRAM
                        # bounce row broadcasts it across partitions 0-63.
                        rb = rb_pool.tile([128, QBS], mybir.dt.float32, tag="rb", name="rb")
                        rbs = rb_pool.tile([8, QBS // 8], mybir.dt.float32, tag="rbs", name="rbs")
                        rbr = rb_pool.tile([8, QBS // 8], mybir.dt.float32, tag="rbr", name="rbr")
                        nc.vector.tensor_copy(rb[64:65, :], o_ps[DK:DK + 1, :])
                        nc.sync.dma_start(out=rbs[:], in_=rb[64:65, :])
                        nc.vector.reciprocal(rbr[:], rbs[:])
                        rrow = rscratch.ap()[qb * H + h: qb * H + h + 1, :]
                        nc.sync.dma_start(out=rrow, in_=rbr[:])
                        nc.sync.dma_start(out=rb[0:64, :],
                                          in_=rrow.to_broadcast([64, QBS]))
                        osm = rb_pool.tile([64, QBS], bf16, tag="osm", bufs=3, name="osm")
                        nc.vector.tensor_mul(osm[:], o_ps[0:DK, :], rb[0:64, :])
                        nc.sync.dma_start(
                            out=o2_sb[(qb, h // 2)][(h % 2) * 64:(h % 2) * 64 + 64, :],
                            in_=osm[:])
                        if dbg and qb == 0 and h == 0:
                            nc.sync.dma_start(out=dbg_rb.ap()[0:1, :], in_=rb[0:1, :])
                            nc.sync.dma_start(out=dbg_rb.ap()[1:2, :], in_=rb[64:65, :])
                            nc.sync.dma_start(out=dbg_osb.ap(), in_=osm[:])

                        # overlap remaining projections with the attention
                        # stream: K(j) fully before head 2j; Q(j) per q-block.
                        steps = ()
                        if qb == 0:
                            steps = [(("k", 1, 0),),
                                     (("k", 1, 1), ("q", 1, 0)),
                                     (("k", 2, 0),), (("k", 2, 1), ("q", 2, 0)),
                                     (("k", 3, 0),), (("k", 3, 1), ("q", 3, 0)),
                                     (("q", 0, 1),), ()][h]
                        else:
                            steps = [(("q", 1, 1),), (("q", 2, 1),),
                                     (("q", 3, 1),)][h] if h < 3 else ()
                        for kind, jj, hh in steps:
                            if kind == "k":
                                k_proj(jj, hh)
                            else:
                                q_proj(jj, hh)
                        if qb == 1 and h == 0:
                            p3(0)


                p3(1)

    nc.compile()
    return nc


def _get_nc():
    if "nc" not in _cache:
        _cache["nc"] = _build_nc()
    return _cache["nc"]


def _make_in_maps(inputs):
    query = np.asarray(inputs["query"], np.float32)
    key = np.asarray(inputs["key"], np.float32)
    value = np.asarray(inputs["value"], np.float32)
    mask = np.asarray(inputs["mask"], bool)
    shared = {
        "wqT": np.ascontiguousarray(np.asarray(inputs["Wq"], np.float32).T).astype(BF16),
        "wkT": np.ascontiguousarray(np.asarray(inputs["Wk"], np.float32).T).astype(BF16),
        "wvT": np.ascontiguousarray(np.asarray(inputs["Wv"], np.float32).T).astype(BF16),
        "woT": np.ascontiguousarray(np.asarray(inputs["Wo"], np.float32).T).astype(BF16),
        "bq": np.asarray(inputs["bq"], np.float32),
        "bk": np.asarray(inputs["bk"], np.float32),
        "bv": np.asarray(inputs["bv"], np.float32),
        "bo": np.asarray(inputs["bo"], np.float32),
    }
    in_maps = []
    for b in range(N_CORES):
        m = dict(shared)
        m["xqT"] = np.ascontiguousarray(query[b].T).astype(BF16)
        m["xkT"] = np.ascontiguousarray(key[b].T).astype(BF16)
        m["xvT"] = np.ascontiguousarray(value[b].T).astype(BF16)
        mb = (~mask[b]).T.astype(BF16)          # (1 - mask)^T, [t2, q]
        m["mbar"] = np.ascontiguousarray(mb.reshape(NCH, 128, T))
        in_maps.append(m)
    return in_maps


def run(inputs, trace=False, **kwargs):
    from concourse.bass_utils import run_bass_kernel_spmd
    nc = _get_nc()
    res = run_bass_kernel_spmd(nc, _make_in_maps(inputs),
                               core_ids=list(range(N_CORES)),
                               trace=trace, **kwargs)
    y = np.stack([np.asarray(res.results[b]["yT"], np.float32).T
                  for b in range(N_CORES)])
    return y, res


def kernel(**inputs) -> np.ndarray:
    y, _ = run(inputs, trace=False)
    return y



# revision 3
# speedup vs baseline: 1.0094x; 1.0094x over previous
"""Multi-head attention (B=8, T=2048, D=512, H=8) on 8 TRN2 NeuronCores.

Sharding: data-parallel over batch - one batch element per core, no
collectives. Host-side prep: transpose x inputs to [D, T], cast matmul
operands to bf16, pass (1 - mask)^T chunk-major; transpose per-core y^T
back to [T, D].

Per-core algorithm (v2, "row-tiled transposed flash"):
  P1: Q^T = Wq x^T and K^T = Wk x^T, both as 4x [128, T] feature-major
      tiles (two heads per tile, NO padding).  V = x Wv^T with a ones
      column per head (denominator trick), chunk-major vaug tiles.
  P2: heads processed in PAIRS (2j, 2j+1).  Per (q-block of 512, pair,
      t2-chunk c):
        S_A^T = K_A^T.T @ Q_A^T   (K=64, PE rows 0-63)   } row-tiled,
        S_B^T = K_B^T.T @ Q_B^T   (K=64, PE rows 64-127) } concurrent
        both into ONE [128, 1024] PSUM tile (A: cols 0-511, B: 512-1023)
        P_raw = exp(S/8)          one ACTIVATE, N=1024
        P     = P_raw * maskbar   one tensor_mul vs a duplicated mask
                                  tile (1/3 of chunks on GpSimd)
        O_aug^T += Vaug_h.T @ P   (M=65; row 64 = softmax denominator)
      epilogue per (qb, pair): denom rows -> split recip -> DRAM-bounce
      broadcast -> normalize straight into o2 (bf16).
  P3: y^T = Wo^T.T @ O^T (+bo) per q-block, interleaved into the next
      q-block's attention stream.

ScalarE exp (256 ACTIVATEs @ ~1.2us) is the roofline; everything else
is scheduled to hide underneath it.
"""

import numpy as np
import ml_dtypes

B, T, FDIM, H = 8, 2048, 512, 8
DK = FDIM // H          # 64
NFT = FDIM // 128       # 4 feature tiles
NCH = T // 128          # 16 t2-chunks
NQB = 4                 # q blocks
QBS = T // NQB          # 512
N_CORES = 8

BF16 = ml_dtypes.bfloat16

_cache = {}


def _build_nc():
    import concourse.bass as bass
    import concourse.mybir as mybir
    from concourse import bacc, tile

    f32 = mybir.dt.float32
    bf16 = mybir.dt.bfloat16
    Exp = mybir.ActivationFunctionType.Exp
    Alu = mybir.AluOpType

    nc = bacc.Bacc("TRN2", target_bir_lowering=False, debug=False,
                   num_devices=N_CORES)

    # DRAM I/O (per-core shard shapes)
    xqT = nc.dram_tensor("xqT", [FDIM, T], bf16, kind="ExternalInput")
    xkT = nc.dram_tensor("xkT", [FDIM, T], bf16, kind="ExternalInput")
    xvT = nc.dram_tensor("xvT", [FDIM, T], bf16, kind="ExternalInput")
    wqT = nc.dram_tensor("wqT", [FDIM, FDIM], bf16, kind="ExternalInput")
    wkT = nc.dram_tensor("wkT", [FDIM, FDIM], bf16, kind="ExternalInput")
    wvT = nc.dram_tensor("wvT", [FDIM, FDIM], bf16, kind="ExternalInput")
    woT = nc.dram_tensor("woT", [FDIM, FDIM], bf16, kind="ExternalInput")
    bq = nc.dram_tensor("bq", [FDIM], f32, kind="ExternalInput")
    bk = nc.dram_tensor("bk", [FDIM], f32, kind="ExternalInput")
    bv = nc.dram_tensor("bv", [FDIM], f32, kind="ExternalInput")
    bo = nc.dram_tensor("bo", [FDIM], f32, kind="ExternalInput")
    mbar = nc.dram_tensor("mbar", [NCH, 128, T], bf16, kind="ExternalInput")
    yT = nc.dram_tensor("yT", [FDIM, T], f32, kind="ExternalOutput")
    # DRAM bounce rows for partition-broadcasting softmax reciprocals
    rscratch = nc.dram_tensor("rscratch", [NQB * H, QBS], f32)

    with tile.TileContext(nc) as tc:
        with (
            tc.tile_pool(name="consts", bufs=1) as consts,
            tc.tile_pool(name="qt", bufs=1) as qt_pool,
            tc.tile_pool(name="kt", bufs=1) as kt_pool,
            tc.tile_pool(name="vaug", bufs=1) as vaug_pool,
            tc.tile_pool(name="osb", bufs=1) as osb_pool,
        ):
            # ---- consts: weights + biases ----
            wq_sb = [consts.tile([128, FDIM], bf16, tag=f"wq{fc}", name=f"wq{fc}") for fc in range(4)]
            wk_sb = [consts.tile([128, FDIM], bf16, tag=f"wk{fc}", name=f"wk{fc}") for fc in range(4)]
            wv_sb = [consts.tile([128, FDIM], bf16, tag=f"wv{fc}", name=f"wv{fc}") for fc in range(4)]
            wo_sb = [consts.tile([128, FDIM], bf16, tag=f"wo{j}", name=f"wo{j}") for j in range(NFT)]
            # wk/wq first: the K/Q projections head the critical path
            for fc in range(4):
                nc.sync.dma_start(out=wk_sb[fc][:], in_=wkT[fc * 128:(fc + 1) * 128, :])
                nc.sync.dma_start(out=wq_sb[fc][:], in_=wqT[fc * 128:(fc + 1) * 128, :])
            for fc in range(4):
                nc.sync.dma_start(out=wv_sb[fc][:], in_=wvT[fc * 128:(fc + 1) * 128, :])
            for j in range(NFT):
                nc.sync.dma_start(out=wo_sb[j][:], in_=woT[j * 128:(j + 1) * 128, :])

            bq_sb = consts.tile([128, NFT], f32, tag="bq", name="bq")
            bk_sb = consts.tile([128, NFT], f32, tag="bk", name="bk")
            bo_sb = consts.tile([128, NFT], f32, tag="bo", name="bo")
            for b_dram, b_t in ((bq, bq_sb), (bk, bk_sb), (bo, bo_sb)):
                nc.sync.dma_start(out=b_t[:], in_=b_dram.ap().rearrange("(j p) -> p j", p=128))
            bv_bcast = consts.tile([128, FDIM], f32, tag="bv_bcast", name="bv_bcast")
            nc.sync.dma_start(
                out=bv_bcast[:],
                in_=bv.ap().rearrange("(a f) -> a f", a=1).to_broadcast([128, FDIM]))

            # ---- persistent activation tiles ----
            qT_sb = [qt_pool.tile([128, T], bf16, tag=f"qT{j}", name=f"qT{j}") for j in range(NFT)]
            kT_sb = [kt_pool.tile([128, T], bf16, tag=f"kT{j}", name=f"kT{j}") for j in range(NFT)]
            vaug = [vaug_pool.tile([128, H * (DK + 1)], bf16, tag=f"va{tt}", name=f"va{tt}")
                    for tt in range(NCH)]
            # ones column per head in V_aug
            for tt in range(NCH):
                va = vaug[tt][:].rearrange("p (h d) -> p h d", d=DK + 1)
                nc.vector.memset(va[:, :, DK:DK + 1], 1.0)

            # o2[j]: rows 0-63 head 2j, rows 64-127 head 2j+1; cols = q
            o2_sb = [osb_pool.tile([128, T], bf16, tag=f"o2_{j}", name=f"o2_{j}")
                     for j in range(NFT)]

            # ============ PSUM (8 banks) ============================
            #  tag "s":  2 x [128,1024] f32 = 4 banks (scores; also
            #            borrowed by QKV projections and P3 y-tiles)
            #  tag "oA"/"oB": 2 x [65,512] each = 4 banks (attnV accum)
            with (
                tc.tile_pool(name="xt", bufs=12) as xt_pool,
                tc.tile_pool(name="mask", bufs=16) as mask_pool,
                tc.tile_pool(name="praw", bufs=3) as praw_pool,
                tc.tile_pool(name="pm", bufs=3) as pm_pool,
                tc.tile_pool(name="epi", bufs=1) as epi_pool,
                tc.tile_pool(name="ysb", bufs=1) as ysb_pool,
                tc.tile_pool(name="psum", bufs=2, space="PSUM") as psum_pool,
            ):
                def load_xT(xT_dram, tag):
                    tiles = []
                    for fc in range(4):
                        xt = xt_pool.tile([128, T], bf16, tag=f"{tag}{fc}", bufs=1, name="xt")
                        nc.sync.dma_start(out=xt[:], in_=xT_dram[fc * 128:(fc + 1) * 128, :])
                        tiles.append(xt)
                    return tiles

                def v_proj_tile(tt):
                    ps = psum_pool.tile([128, 512], f32, tag="s", name="vp")
                    for fc in range(4):
                        nc.tensor.matmul(
                            ps[:],
                            xts_v[fc][:, tt * 128:(tt + 1) * 128],
                            wv_sb[fc][:],
                            start=(fc == 0), stop=(fc == 3),
                        )
                    va = vaug[tt][:].rearrange("p (h d) -> p h d", d=DK + 1)
                    nc.vector.scalar_tensor_tensor(
                        out=va[:, :, 0:DK],
                        in0=ps[:].rearrange("p (h d) -> p h d", d=DK),
                        scalar=1.0,
                        in1=bv_bcast[:].rearrange("p (h d) -> p h d", d=DK),
                        op0=Alu.mult, op1=Alu.add,
                    )

                def proj_group(j, xts, w_sb, b_t, dst, s):
                    ps = psum_pool.tile([128, 512], f32, tag="s", name="qkp")
                    for fc in range(4):
                        nc.tensor.matmul(
                            ps[:],
                            w_sb[fc][:, j * 128:(j + 1) * 128],
                            xts[fc][:, s * 512:(s + 1) * 512],
                            start=(fc == 0), stop=(fc == 3),
                        )
                    nc.vector.tensor_scalar_add(
                        dst[:, s * 512:(s + 1) * 512], ps[:], b_t[:, j:j + 1])

                def p3_unit(qb, i):
                    qsl = slice(qb * QBS, (qb + 1) * QBS)
                    y_ps = psum_pool.tile([128, 512], f32, tag="s", name="y")
                    for j in range(NFT):
                        nc.tensor.matmul(
                            y_ps[:],
                            wo_sb[j][:, i * 128:(i + 1) * 128],
                            o2_sb[j][:, qsl],
                            start=(j == 0), stop=(j == NFT - 1),
                        )
                    y_sb = ysb_pool.tile([128, 512], f32, tag="ysb", bufs=2, name="ysb")
                    nc.vector.tensor_scalar_add(y_sb[:], y_ps[:], bo_sb[:, i:i + 1])
                    nc.sync.dma_start(out=yT[i * 128:(i + 1) * 128, qsl], in_=y_sb[:])

                # ---- P1 prefix: stage inputs, project K0/Q0 + V0/V1 ----
                xts_k = load_xT(xkT, "xk")
                xts_q = load_xT(xqT, "xq")
                xts_v = load_xT(xvT, "xv")
                for s in range(4):
                    proj_group(0, xts_k, wk_sb, bk_sb, kT_sb[0], s)
                for s in range(4):
                    proj_group(0, xts_q, wq_sb, bq_sb, qT_sb[0], s)
                v_proj_tile(0)
                v_proj_tile(1)

                # mask tiles for qb0 (each tile: chunk mask duplicated 2x)
                mask_t = {}

                def mask_dma(qb, c):
                    mt = mask_pool.tile([128, 1024], bf16, tag=f"mk{c}", bufs=1,
                                        name="mask")
                    qsl = slice(qb * QBS, (qb + 1) * QBS)
                    nc.sync.dma_start(out=mt[:, 0:512], in_=mbar[c, :, qsl])
                    nc.sync.dma_start(out=mt[:, 512:1024], in_=mbar[c, :, qsl])
                    mask_t[c] = mt

                for c in range(NCH):
                    mask_dma(0, c)

                # extra-work schedule: (qb, pair) -> list of (c, fn) lambdas
                def extra_steps(qb, pair):
                    steps = {}
                    if qb == 0 and pair == 0:
                        for c in range(14):
                            steps.setdefault(c, []).append(
                                lambda tt=c + 2: v_proj_tile(tt))
                        for idx, c in enumerate((1, 4, 7, 10)):
                            steps.setdefault(c, []).append(
                                lambda s=idx: proj_group(1, xts_k, wk_sb, bk_sb, kT_sb[1], s))
                        for idx, c in enumerate((2, 5, 8, 11)):
                            steps.setdefault(c, []).append(
                                lambda s=idx: proj_group(1, xts_q, wq_sb, bq_sb, qT_sb[1], s))
                    elif qb == 0 and pair in (1, 2):
                        jj = pair + 1
                        for idx, c in enumerate((0, 2, 4, 6)):
                            steps.setdefault(c, []).append(
                                lambda s=idx, j=jj: proj_group(j, xts_k, wk_sb, bk_sb, kT_sb[j], s))
                        for idx, c in enumerate((8, 10, 12, 14)):
                            steps.setdefault(c, []).append(
                                lambda s=idx, j=jj: proj_group(j, xts_q, wq_sb, bq_sb, qT_sb[j], s))
                    elif qb > 0:
                        # one P3 unit of the previous q-block per pair
                        steps[8] = [lambda i=pair, q=qb - 1: p3_unit(q, i)]
                    return steps

                def epilogue(qb, pair, oA, oB):
                    pp = qb * 4 + pair
                    qsl = slice(qb * QBS, (qb + 1) * QBS)
                    # DVE lanes cannot move data across partitions: copy the
                    # denominator rows (partition 64) into an aligned SBUF
                    # tile, then DMA does the partition reshapes/broadcasts.
                    dn = epi_pool.tile([65, 1024], f32, tag="dn", bufs=2, name="dn")
                    nc.vector.tensor_copy(dn[64:65, 0:512], oA[64:65, :])
                    nc.vector.tensor_copy(dn[64:65, 512:1024], oB[64:65, :])
                    rbs = epi_pool.tile([64, 16], f32, tag="rbs", bufs=2, name="rbs")
                    nc.sync.dma_start(out=rbs[0:32, :], in_=dn[64:65, 0:512])
                    nc.sync.dma_start(out=rbs[32:64, :], in_=dn[64:65, 512:1024])
                    rbr = epi_pool.tile([64, 16], f32, tag="rbr", bufs=2, name="rbr")
                    nc.vector.reciprocal(rbr[:], rbs[:])
                    rowA = rscratch.ap()[2 * pp:2 * pp + 1, :]
                    rowB = rscratch.ap()[2 * pp + 1:2 * pp + 2, :]
                    nc.sync.dma_start(out=rowA, in_=rbr[0:32, :])
                    nc.sync.dma_start(out=rowB, in_=rbr[32:64, :])
                    rbA = epi_pool.tile([64, 512], f32, tag="rb", bufs=4, name="rb")
                    rbB = epi_pool.tile([64, 512], f32, tag="rb", bufs=4, name="rb")
                    nc.sync.dma_start(out=rbA[:], in_=rowA.to_broadcast([64, 512]))
                    nc.sync.dma_start(out=rbB[:], in_=rowB.to_broadcast([64, 512]))
                    # head A lands on partitions 0-63 directly; head B needs a
                    # DMA partition-shift to o2 rows 64-127
                    nc.vector.tensor_mul(o2_sb[pair][0:64, qsl], oA[0:64, :], rbA[:])
                    osmB = epi_pool.tile([64, 512], bf16, tag="osm", bufs=2, name="osm")
                    nc.vector.tensor_mul(osmB[:], oB[0:64, :], rbB[:])
                    nc.sync.dma_start(out=o2_sb[pair][64:128, qsl], in_=osmB[:])

                # ---- P2 main loop ----
                for qb in range(NQB):
                    qsl = slice(qb * QBS, (qb + 1) * QBS)
                    for pair in range(NQB):
                        hA, hB = 2 * pair, 2 * pair + 1
                        steps = extra_steps(qb, pair)
                        oA = psum_pool.tile([DK + 1, 512], f32, tag="oA",
                                            bufs=2, name="oA")
                        oB = psum_pool.tile([DK + 1, 512], f32, tag="oB",
                                            bufs=2, name="oB")
                        p_ms = {}
                        for c in range(NCH + 1):
                            if c < NCH:
                                s_t = psum_pool.tile([128, 1024], f32, tag="s",
                                                     name="s")
                                nc.tensor.matmul(
                                    s_t[:, 0:512],
                                    kT_sb[pair][0:64, c * 128:(c + 1) * 128],
                                    qT_sb[pair][0:64, qsl],
                                    start=True, stop=True,
                                )
                                nc.tensor.matmul(
                                    s_t[:, 512:1024],
                                    kT_sb[pair][64:128, c * 128:(c + 1) * 128],
                                    qT_sb[pair][64:128, qsl],
                                    start=True, stop=True,
                                )
                                p_raw = praw_pool.tile([128, 1024], bf16,
                                                       tag="praw", name="praw")
                                nc.scalar.activation(p_raw[:], s_t[:], Exp,
                                                     bias=0.0, scale=0.125)
                                p_m = pm_pool.tile([128, 1024], bf16, tag="pm",
                                                   name="pm")
                                eng = nc.gpsimd if c in (2, 5, 8, 11) else nc.vector
                                eng.tensor_mul(p_m[:], p_raw[:], mask_t[c][:])
                                p_ms[c] = p_m
                            # attnV one chunk behind so a late p_m never
                            # blocks the next scores matmul in the PE queue
                            if c >= 1:
                                cc = c - 1
                                nc.tensor.matmul(
                                    oA[:],
                                    vaug[cc][:, hA * (DK + 1):(hA + 1) * (DK + 1)],
                                    p_ms[cc][:, 0:512],
                                    start=(cc == 0), stop=(cc == NCH - 1),
                                )
                                nc.tensor.matmul(
                                    oB[:],
                                    vaug[cc][:, hB * (DK + 1):(hB + 1) * (DK + 1)],
                                    p_ms[cc][:, 512:1024],
                                    start=(cc == 0), stop=(cc == NCH - 1),
                                )
                                del p_ms[cc]
                            for fn in steps.get(c, ()):
                                fn()
                        # refresh mask tiles for the next (qb, pair-0)
                        if pair == 3 and qb < NQB - 1:
                            for c in range(NCH):
                                mask_dma(qb + 1, c)
                        epilogue(qb, pair, oA, oB)

                # final output projection for the last q-block
                for i in range(NFT):
                    p3_unit(NQB - 1, i)

    nc.compile()
    return nc


def _get_nc():
    if "nc" not in _cache:
        _cache["nc"] = _build_nc()
    return _cache["nc"]


def _make_in_maps(inputs):
    query = np.asarray(inputs["query"], np.float32)
    key = np.asarray(inputs["key"], np.float32)
    value = np.asarray(inputs["value"], np.float32)
    mask = np.asarray(inputs["mask"], bool)
    shared = {
        "wqT": np.ascontiguousarray(np.asarray(inputs["Wq"], np.float32).T).astype(BF16),
        "wkT": np.ascontiguousarray(np.asarray(inputs["Wk"], np.float32).T).astype(BF16),
        "wvT": np.ascontiguousarray(np.asarray(inputs["Wv"], np.float32).T).astype(BF16),
        "woT": np.ascontiguousarray(np.asarray(inputs["Wo"], np.float32).T).astype(BF16),
        "bq": np.asarray(inputs["bq"], np.float32),
        "bk": np.asarray(inputs["bk"], np.float32),
        "bv": np.asarray(inputs["bv"], np.float32),
        "bo": np.asarray(inputs["bo"], np.float32),
    }
    in_maps = []
    for b in range(N_CORES):
        m = dict(shared)
        m["xqT"] = np.ascontiguousarray(query[b].T).astype(BF16)
        m["xkT"] = np.ascontiguousarray(key[b].T).astype(BF16)
        m["xvT"] = np.ascontiguousarray(value[b].T).astype(BF16)
        mb = (~mask[b]).T.astype(BF16)          # (1 - mask)^T, [t2, q]
        m["mbar"] = np.ascontiguousarray(mb.reshape(NCH, 128, T))
        in_maps.append(m)
    return in_maps


def run(inputs, trace=False, **kwargs):
    from concourse.bass_utils import run_bass_kernel_spmd
    nc = _get_nc()
    res = run_bass_kernel_spmd(nc, _make_in_maps(inputs),
                               core_ids=list(range(N_CORES)),
                               trace=trace, **kwargs)
    y = np.stack([np.asarray(res.results[b]["yT"], np.float32).T
                  for b in range(N_CORES)])
    return y, res


def kernel(**inputs) -> np.ndarray:
    y, _ = run(inputs, trace=False)
    return y


# revision 8
# speedup vs baseline: 1.1588x; 1.1479x over previous
"""Multi-head attention (B=8, T=2048, D=512, H=8) on 8 TRN2 NeuronCores.

Sharding: data-parallel over batch - one batch element per core, no
collectives. Host-side prep: transpose x inputs to [D, T], cast matmul
operands to bf16, pass (1 - mask)^T chunk-major; transpose per-core y^T
back to [T, D].

Per-core algorithm (v2, "row-tiled transposed flash"):
  P1: Q^T = Wq x^T and K^T = Wk x^T, both as 4x [128, T] feature-major
      tiles (two heads per tile, NO padding).  V = x Wv^T with a ones
      column per head (denominator trick), chunk-major vaug tiles.
  P2: heads processed in PAIRS (2j, 2j+1).  Per (q-block of 512, pair,
      t2-chunk c):
        S_A^T = K_A^T.T @ Q_A^T   (K=64, PE rows 0-63)   } row-tiled,
        S_B^T = K_B^T.T @ Q_B^T   (K=64, PE rows 64-127) } concurrent
        both into ONE [128, 1024] PSUM tile (A: cols 0-511, B: 512-1023)
        P_raw = exp(S/8)          one ACTIVATE, N=1024
        P     = P_raw * maskbar   one tensor_mul vs a duplicated mask
                                  tile (1/3 of chunks on GpSimd)
        O_aug^T += Vaug_h.T @ P   (M=65; row 64 = softmax denominator)
      epilogue per (qb, pair): denom rows -> split recip -> DRAM-bounce
      broadcast -> normalize straight into o2 (bf16).
  P3: y^T = Wo^T.T @ O^T (+bo) per q-block, interleaved into the next
      q-block's attention stream.

ScalarE exp (256 ACTIVATEs @ ~1.2us) is the roofline; everything else
is scheduled to hide underneath it.
"""

import numpy as np
import ml_dtypes

B, T, FDIM, H = 8, 2048, 512, 8
DK = FDIM // H          # 64
NFT = FDIM // 128       # 4 feature tiles
NCH = T // 128          # 16 t2-chunks
NQB = 4                 # q blocks
QBS = T // NQB          # 512
N_CORES = 8

BF16 = ml_dtypes.bfloat16

_cache = {}


def _build_nc():
    import concourse.bass as bass
    import concourse.mybir as mybir
    from concourse import bacc, tile

    f32 = mybir.dt.float32
    bf16 = mybir.dt.bfloat16
    Exp = mybir.ActivationFunctionType.Exp
    Alu = mybir.AluOpType

    nc = bacc.Bacc("TRN2", target_bir_lowering=False, debug=False,
                   num_devices=N_CORES)

    # DRAM I/O (per-core shard shapes)
    xqT = nc.dram_tensor("xqT", [FDIM, T], bf16, kind="ExternalInput")
    xkT = nc.dram_tensor("xkT", [FDIM, T], bf16, kind="ExternalInput")
    xvT = nc.dram_tensor("xvT", [FDIM, T], bf16, kind="ExternalInput")
    wqT = nc.dram_tensor("wqT", [FDIM, FDIM], bf16, kind="ExternalInput")
    wkT = nc.dram_tensor("wkT", [FDIM, FDIM], bf16, kind="ExternalInput")
    wvT = nc.dram_tensor("wvT", [FDIM, FDIM], bf16, kind="ExternalInput")
    woT = nc.dram_tensor("woT", [FDIM, FDIM], bf16, kind="ExternalInput")
    bq = nc.dram_tensor("bq", [FDIM], f32, kind="ExternalInput")
    bk = nc.dram_tensor("bk", [FDIM], f32, kind="ExternalInput")
    bv = nc.dram_tensor("bv", [FDIM], f32, kind="ExternalInput")
    bo = nc.dram_tensor("bo", [FDIM], f32, kind="ExternalInput")
    mbar = nc.dram_tensor("mbar", [NCH, 128, T], bf16, kind="ExternalInput")
    yT = nc.dram_tensor("yT", [FDIM, T], f32, kind="ExternalOutput")
    # DRAM bounce rows for partition-broadcasting softmax reciprocals
    rscratch = nc.dram_tensor("rscratch", [NQB * H, QBS], f32)

    with tile.TileContext(nc) as tc:
        with (
            tc.tile_pool(name="consts", bufs=1) as consts,
            tc.tile_pool(name="qt", bufs=1) as qt_pool,
            tc.tile_pool(name="kt", bufs=1) as kt_pool,
            tc.tile_pool(name="vaug", bufs=1) as vaug_pool,
            tc.tile_pool(name="osb", bufs=1) as osb_pool,
        ):
            # ---- consts: weights + biases ----
            wq_sb = [consts.tile([128, FDIM], bf16, tag=f"wq{fc}", name=f"wq{fc}") for fc in range(4)]
            wk_sb = [consts.tile([128, FDIM], bf16, tag=f"wk{fc}", name=f"wk{fc}") for fc in range(4)]
            wv_sb = [consts.tile([128, FDIM], bf16, tag=f"wv{fc}", name=f"wv{fc}") for fc in range(4)]
            wo_sb = [consts.tile([128, FDIM], bf16, tag=f"wo{j}", name=f"wo{j}") for j in range(NFT)]
            # wk/wq first: the K/Q projections head the critical path.
            # wo is DMA'd much later (first needed ~100us in).
            for fc in range(4):
                nc.sync.dma_start(out=wk_sb[fc][:], in_=wkT[fc * 128:(fc + 1) * 128, :])
                nc.sync.dma_start(out=wq_sb[fc][:], in_=wqT[fc * 128:(fc + 1) * 128, :])
            for fc in range(4):
                nc.sync.dma_start(out=wv_sb[fc][:], in_=wvT[fc * 128:(fc + 1) * 128, :])

            bq_sb = consts.tile([128, NFT], f32, tag="bq", name="bq")
            bk_sb = consts.tile([128, NFT], f32, tag="bk", name="bk")
            bo_sb = consts.tile([128, NFT], f32, tag="bo", name="bo")
            for b_dram, b_t in ((bq, bq_sb), (bk, bk_sb), (bo, bo_sb)):
                nc.sync.dma_start(out=b_t[:], in_=b_dram.ap().rearrange("(j p) -> p j", p=128))
            bv_bcast = consts.tile([128, FDIM], f32, tag="bv_bcast", name="bv_bcast")
            nc.sync.dma_start(
                out=bv_bcast[:],
                in_=bv.ap().rearrange("(a f) -> a f", a=1).to_broadcast([128, FDIM]))

            # ---- persistent activation tiles ----
            qT_sb = [qt_pool.tile([128, T], bf16, tag=f"qT{j}", name=f"qT{j}") for j in range(NFT)]
            kT_sb = [kt_pool.tile([128, T], bf16, tag=f"kT{j}", name=f"kT{j}") for j in range(NFT)]
            vaug = [vaug_pool.tile([128, H * (DK + 1)], bf16, tag=f"va{tt}", name=f"va{tt}")
                    for tt in range(NCH)]
            # ones column per head in V_aug
            for tt in range(NCH):
                va = vaug[tt][:].rearrange("p (h d) -> p h d", d=DK + 1)
                nc.vector.memset(va[:, :, DK:DK + 1], 1.0)

            # o2[j]: rows 0-63 head 2j, rows 64-127 head 2j+1; cols = q
            o2_sb = [osb_pool.tile([128, T], bf16, tag=f"o2_{j}", name=f"o2_{j}")
                     for j in range(NFT)]

            # ============ PSUM (8 banks) ============================
            #  tag "s":  2 x [128,1024] f32 = 4 banks (scores; also
            #            borrowed by QKV projections and P3 y-tiles)
            #  tag "oA"/"oB": 2 x [65,512] each = 4 banks (attnV accum)
            with (
                tc.tile_pool(name="xt", bufs=12) as xt_pool,
                tc.tile_pool(name="mask", bufs=16) as mask_pool,
                tc.tile_pool(name="praw", bufs=3) as praw_pool,
                tc.tile_pool(name="pm", bufs=3) as pm_pool,
                tc.tile_pool(name="epi", bufs=1) as epi_pool,
                tc.tile_pool(name="ysb", bufs=1) as ysb_pool,
                tc.tile_pool(name="psum", bufs=2, space="PSUM") as psum_pool,
            ):
                def alloc_xT(tag):
                    return [xt_pool.tile([128, T], bf16, tag=f"{tag}{fc}", bufs=1,
                                         name="xt") for fc in range(4)]

                def load_xT_quarter(xT_dram, tiles, s):
                    # one column-quarter of all 4 feature tiles, so the
                    # matching projection group can start 4x earlier
                    sl = slice(s * 512, (s + 1) * 512)
                    for fc in range(4):
                        nc.sync.dma_start(out=tiles[fc][:, sl],
                                          in_=xT_dram[fc * 128:(fc + 1) * 128, sl])

                def v_proj_tile(tt):
                    ps = psum_pool.tile([128, 512], f32, tag="s", name="vp")
                    for fc in range(4):
                        nc.tensor.matmul(
                            ps[:],
                            xts_v[fc][:, tt * 128:(tt + 1) * 128],
                            wv_sb[fc][:],
                            start=(fc == 0), stop=(fc == 3),
                        )
                    va = vaug[tt][:].rearrange("p (h d) -> p h d", d=DK + 1)
                    nc.vector.scalar_tensor_tensor(
                        out=va[:, :, 0:DK],
                        in0=ps[:].rearrange("p (h d) -> p h d", d=DK),
                        scalar=1.0,
                        in1=bv_bcast[:].rearrange("p (h d) -> p h d", d=DK),
                        op0=Alu.mult, op1=Alu.add,
                    )

                def proj_group(j, xts, w_sb, b_t, dst, s):
                    ps = psum_pool.tile([128, 512], f32, tag="s", name="qkp")
                    for fc in range(4):
                        nc.tensor.matmul(
                            ps[:],
                            w_sb[fc][:, j * 128:(j + 1) * 128],
                            xts[fc][:, s * 512:(s + 1) * 512],
                            start=(fc == 0), stop=(fc == 3),
                        )
                    nc.vector.tensor_scalar_add(
                        dst[:, s * 512:(s + 1) * 512], ps[:], b_t[:, j:j + 1])

                def p3_unit(qb, i):
                    qsl = slice(qb * QBS, (qb + 1) * QBS)
                    y_ps = psum_pool.tile([128, 512], f32, tag="s", name="y")
                    for j in range(NFT):
                        nc.tensor.matmul(
                            y_ps[:],
                            wo_sb[j][:, i * 128:(i + 1) * 128],
                            o2_sb[j][:, qsl],
                            start=(j == 0), stop=(j == NFT - 1),
                        )
                    y_sb = ysb_pool.tile([128, 512], f32, tag="ysb", bufs=2, name="ysb")
                    nc.vector.tensor_scalar_add(y_sb[:], y_ps[:], bo_sb[:, i:i + 1])
                    nc.sync.dma_start(out=yT[i * 128:(i + 1) * 128, qsl], in_=y_sb[:])

                # ---- P1 prefix ------------------------------------------
                # DMA order is the head critical path: wk, xk-s0, wq, xq-s0,
                # wv, xv-s0 and the first mask tiles come first; everything
                # else streams in behind while compute already runs.
                xts_k = alloc_xT("xk")
                xts_q = alloc_xT("xq")
                xts_v = alloc_xT("xv")

                mask_t = {}

                def mask_dma(qb, c):
                    mt = mask_pool.tile([128, 1024], bf16, tag=f"mk{c}", bufs=1,
                                        name="mask")
                    qsl = slice(qb * QBS, (qb + 1) * QBS)
                    nc.sync.dma_start(out=mt[:, 0:512], in_=mbar[c, :, qsl])
                    nc.sync.dma_start(out=mt[:, 512:1024], in_=mbar[c, :, qsl])
                    mask_t[c] = mt

                load_xT_quarter(xkT, xts_k, 0)
                load_xT_quarter(xqT, xts_q, 0)
                load_xT_quarter(xvT, xts_v, 0)
                mask_dma(0, 0)
                mask_dma(0, 1)
                load_xT_quarter(xkT, xts_k, 1)
                mask_dma(0, 2)
                mask_dma(0, 3)
                load_xT_quarter(xvT, xts_v, 1)
                load_xT_quarter(xkT, xts_k, 2)
                load_xT_quarter(xkT, xts_k, 3)
                load_xT_quarter(xvT, xts_v, 2)
                load_xT_quarter(xvT, xts_v, 3)
                for c in range(4, NCH):
                    mask_dma(0, c)
                for s in range(1, 4):
                    load_xT_quarter(xqT, xts_q, s)
                for j in range(NFT):
                    nc.sync.dma_start(out=wo_sb[j][:], in_=woT[j * 128:(j + 1) * 128, :])

                # q/k projection groups: K(j) group s covers t2 chunks
                # 4s..4s+3 (all needed by pair j of every q-block); Q(j)
                # group s is only needed once q-block s is reached.
                proj_k = lambda j, s: proj_group(j, xts_k, wk_sb, bk_sb, kT_sb[j], s)
                proj_q = lambda j, s: proj_group(j, xts_q, wq_sb, bq_sb, qT_sb[j], s)

                proj_k(0, 0)
                proj_q(0, 0)
                v_proj_tile(0)

                # extra-work schedule: (qb, pair) -> {c: [fns]}
                def extra_steps(qb, pair):
                    steps = {}
                    add = lambda c, fn: steps.setdefault(c, []).append(fn)
                    if qb == 0:
                        if pair == 0:
                            for c in range(3):
                                add(c, lambda s=c + 1: proj_k(0, s))
                            for c in range(15):
                                add(c, lambda tt=c + 1: v_proj_tile(tt))
                            for idx, c in enumerate((3, 6, 9, 12)):
                                add(c, lambda s=idx: proj_k(1, s))
                            add(10, lambda: proj_q(1, 0))
                        elif pair in (1, 2):
                            jj = pair + 1
                            for idx, c in enumerate((0, 4, 8, 12)):
                                add(c, lambda s=idx, j=jj: proj_k(j, s))
                            add(10, lambda j=jj: proj_q(j, 0))
                        else:
                            for j in range(4):
                                add(1 + 4 * j, lambda j=j: proj_q(j, 1))
                    else:
                        # one P3 unit of the previous q-block per pair, plus
                        # the Q projection slice needed two q-blocks ahead
                        add(8, lambda i=pair, q=qb - 1: p3_unit(q, i))
                        if qb < 3:
                            add(3, lambda j=pair, s=qb + 1: proj_q(j, s))
                    return steps

                def epilogue(qb, pair, oA, oB):
                    pp = qb * 4 + pair
                    qsl = slice(qb * QBS, (qb + 1) * QBS)
                    # DVE lanes cannot move data across partitions: copy the
                    # denominator rows (partition 64) into an aligned SBUF
                    # tile, then DMA does the partition reshapes/broadcasts.
                    dn = epi_pool.tile([65, 1024], f32, tag="dn", bufs=2, name="dn")
                    nc.vector.tensor_copy(dn[64:65, 0:512], oA[64:65, :])
                    nc.vector.tensor_copy(dn[64:65, 512:1024], oB[64:65, :])
                    rbs = epi_pool.tile([64, 16], f32, tag="rbs", bufs=2, name="rbs")
                    nc.sync.dma_start(out=rbs[0:32, :], in_=dn[64:65, 0:512])
                    nc.sync.dma_start(out=rbs[32:64, :], in_=dn[64:65, 512:1024])
                    rbr = epi_pool.tile([64, 16], f32, tag="rbr", bufs=2, name="rbr")
                    nc.vector.reciprocal(rbr[:], rbs[:])
                    rowA = rscratch.ap()[2 * pp:2 * pp + 1, :]
                    rowB = rscratch.ap()[2 * pp + 1:2 * pp + 2, :]
                    nc.sync.dma_start(out=rowA, in_=rbr[0:32, :])
                    nc.sync.dma_start(out=rowB, in_=rbr[32:64, :])
                    rbA = epi_pool.tile([64, 512], f32, tag="rb", bufs=4, name="rb")
                    rbB = epi_pool.tile([64, 512], f32, tag="rb", bufs=4, name="rb")
                    nc.sync.dma_start(out=rbA[:], in_=rowA.to_broadcast([64, 512]))
                    nc.sync.dma_start(out=rbB[:], in_=rowB.to_broadcast([64, 512]))
                    # head A lands on partitions 0-63 directly; head B needs a
                    # DMA partition-shift to o2 rows 64-127
                    nc.vector.tensor_mul(o2_sb[pair][0:64, qsl], oA[0:64, :], rbA[:])
                    osmB = epi_pool.tile([64, 512], bf16, tag="osm", bufs=2, name="osm")
                    nc.vector.tensor_mul(osmB[:], oB[0:64, :], rbB[:])
                    nc.sync.dma_start(out=o2_sb[pair][64:128, qsl], in_=osmB[:])

                # ---- P2 main loop ----
                SKEW = 3          # attnV trails scores/exp by 3 chunks so a
                                  # slow (GpSimd) mask-mul never head-of-line
                                  # blocks the PE queue
                GP_CHUNKS = (3, 8, 13)
                for qb in range(NQB):
                    qsl = slice(qb * QBS, (qb + 1) * QBS)
                    for pair in range(NQB):
                        hA, hB = 2 * pair, 2 * pair + 1
                        steps = extra_steps(qb, pair)
                        oA = psum_pool.tile([DK + 1, 512], f32, tag="oA",
                                            bufs=2, name="oA")
                        oB = psum_pool.tile([DK + 1, 512], f32, tag="oB",
                                            bufs=2, name="oB")
                        p_ms = {}
                        for c in range(NCH + SKEW):
                            if c < NCH:
                                s_t = psum_pool.tile([128, 1024], f32, tag="s",
                                                     name="s")
                                nc.tensor.matmul(
                                    s_t[:, 0:512],
                                    kT_sb[pair][0:64, c * 128:(c + 1) * 128],
                                    qT_sb[pair][0:64, qsl],
                                    start=True, stop=True,
                                )
                                nc.tensor.matmul(
                                    s_t[:, 512:1024],
                                    kT_sb[pair][64:128, c * 128:(c + 1) * 128],
                                    qT_sb[pair][64:128, qsl],
                                    start=True, stop=True,
                                )
                                p_raw = praw_pool.tile([128, 1024], bf16,
                                                       tag="praw", bufs=4,
                                                       name="praw")
                                nc.scalar.activation(p_raw[:], s_t[:], Exp,
                                                     bias=0.0, scale=0.125)
                                p_m = pm_pool.tile([128, 1024], bf16, tag="pm",
                                                   bufs=5, name="pm")
                                eng = nc.gpsimd if c in GP_CHUNKS else nc.vector
                                eng.tensor_mul(p_m[:], p_raw[:], mask_t[c][:])
                                p_ms[c] = p_m
                            if c >= SKEW:
                                cc = c - SKEW
                                nc.tensor.matmul(
                                    oA[:],
                                    vaug[cc][:, hA * (DK + 1):(hA + 1) * (DK + 1)],
                                    p_ms[cc][:, 0:512],
                                    start=(cc == 0), stop=(cc == NCH - 1),
                                )
                                nc.tensor.matmul(
                                    oB[:],
                                    vaug[cc][:, hB * (DK + 1):(hB + 1) * (DK + 1)],
                                    p_ms[cc][:, 512:1024],
                                    start=(cc == 0), stop=(cc == NCH - 1),
                                )
                                del p_ms[cc]
                            for fn in steps.get(c, ()):
                                fn()
                            # refresh one mask tile per iteration during the
                            # last pair (spread out the DMA burst)
                            if pair == 3 and qb < NQB - 1 and c < NCH:
                                mask_dma(qb + 1, c)
                        epilogue(qb, pair, oA, oB)

                # final output projection for the last q-block
                for i in range(NFT):
                    p3_unit(NQB - 1, i)

    nc.compile()
    return nc


def _get_nc():
    if "nc" not in _cache:
        _cache["nc"] = _build_nc()
    return _cache["nc"]


def _make_in_maps(inputs):
    query = np.asarray(inputs["query"], np.float32)
    key = np.asarray(inputs["key"], np.float32)
    value = np.asarray(inputs["value"], np.float32)
    mask = np.asarray(inputs["mask"], bool)
    shared = {
        "wqT": np.ascontiguousarray(np.asarray(inputs["Wq"], np.float32).T).astype(BF16),
        "wkT": np.ascontiguousarray(np.asarray(inputs["Wk"], np.float32).T).astype(BF16),
        "wvT": np.ascontiguousarray(np.asarray(inputs["Wv"], np.float32).T).astype(BF16),
        "woT": np.ascontiguousarray(np.asarray(inputs["Wo"], np.float32).T).astype(BF16),
        "bq": np.asarray(inputs["bq"], np.float32),
        "bk": np.asarray(inputs["bk"], np.float32),
        "bv": np.asarray(inputs["bv"], np.float32),
        "bo": np.asarray(inputs["bo"], np.float32),
    }
    in_maps = []
    for b in range(N_CORES):
        m = dict(shared)
        m["xqT"] = np.ascontiguousarray(query[b].T).astype(BF16)
        m["xkT"] = np.ascontiguousarray(key[b].T).astype(BF16)
        m["xvT"] = np.ascontiguousarray(value[b].T).astype(BF16)
        mb = (~mask[b]).T.astype(BF16)          # (1 - mask)^T, [t2, q]
        m["mbar"] = np.ascontiguousarray(mb.reshape(NCH, 128, T))
        in_maps.append(m)
    return in_maps


def run(inputs, trace=False, **kwargs):
    from concourse.bass_utils import run_bass_kernel_spmd
    nc = _get_nc()
    res = run_bass_kernel_spmd(nc, _make_in_maps(inputs),
                               core_ids=list(range(N_CORES)),
                               trace=trace, **kwargs)
    y = np.stack([np.asarray(res.results[b]["yT"], np.float32).T
                  for b in range(N_CORES)])
    return y, res


def kernel(**inputs) -> np.ndarray:
    y, _ = run(inputs, trace=False)
    return y
